# revision 66
# baseline (speedup 1.0000x reference)
"""Trainium2 Bass kernel for nn_DecoderBlock (B=4, S=2048, E=1024, H=16, D=64).

Sharding: 8 cores = 4 batches x 2 sequence-halves. Each core owns 1024 query
positions of one batch (a balanced causal split: core-even takes q [0:512)+
[1536:2048), core-odd takes q [512:1536)) and recomputes full-S K/V for its
batch locally (no collectives). Proj + FFN are token-parallel on the owned
1024 positions. Everything on-chip is in transposed layout (feature dim on
partitions); the host pre-transposes x and re-transposes the output.

The per-core program is identical (SPMD); per-core differences (which q
columns, causal masks) are encoded in the host-prepared inputs: xT columns
are reordered to [own-q | other-q], and causal masks are shipped per-core.
"""

import numpy as np
import ml_dtypes

import concourse.bass as bass
import concourse.tile as tile
from concourse import bacc, mybir
from concourse.bass_utils import run_bass_kernel_spmd

B, S, E, H, D = 4, 2048, 1024, 16, 64
QC = 1024          # queries owned per core
CH = 512           # q-chunk (matmul moving dim)
ET = E // 128      # 8 e-tiles
HT = (4 * E) // 128  # 32 ffn hidden tiles
SCALE = float(E) ** -0.5

F32R = mybir.dt.float32r
F32 = mybir.dt.float32
BF16 = mybir.dt.bfloat16
F8 = mybir.dt.float8e4
F8E5 = mybir.dt.float8e5
NPF8 = ml_dtypes.float8_e4m3
NPF8E5 = ml_dtypes.float8_e5m2
DRM = mybir.MatmulPerfMode.DoubleRow

# fp8 quantization scales (all powers of 2; dequants folded into drains)
S_X = 8.0       # x (QKV input)
S_W = 512.0     # wq/wk/wv
S_X1 = 16.0     # ffn input x1
S_W1 = 512.0    # w1
S_H = 16.0      # ffn hidden h
S_W2 = 1024.0   # w2
C_QK = 2.0 ** -12   # 1/(S_X*S_W): q/k/v psum dequant
C_H = 2.0 ** -9     # S_H/(S_X1*S_W1): ffn1 psum -> h_hi scale
C_F2 = 2.0 ** -14   # 1/(S_H*S_W2): ffn2 psum dequant

# Chunk-A slot table: (t_tile, diag_mask_idx or None); uniform across cores.
# xT t-order is [own qA | own qB | other qA | other qB] (512 cols each).
# Slots with mask None are all-valid or all-invalid depending on the core's
# half; the per-core exp-bias table (eb) turns invalid ones into es=0.
# Chunk B is handled as 8 hardcoded t-tile pairs in _attend (fp8 DoubleRow);
# pairs 2,3 carry the diagonal masks, pairs 6,7 are all-or-none via bias.
CHUNK_A = [(0, 0), (1, 1), (2, 2), (3, 3),
           (8, None), (9, None), (10, None), (11, None)]
N_MASKS = 4
LN_SES = 2.772588722239781  # ln(16): folds the es fp8 scale into exp

_CACHE = {}
LAST_RESULTS = None


def _mm3(nc, ps, whi, wlo, xhi, xlo, n_slice, terms=3):
    """3-term hi/lo fp8 DoubleRow accumulation: (Whi+Wlo)^T Xhi + Whi^T Xlo.

    whi/wlo: [128, ET, M] stationary; xhi/xlo: [128, ET, S] moving, column
    slice n_slice. K=E contracted as 4 DoubleRow steps of 256."""
    seq = [(whi, xhi), (wlo, xhi), (whi, xlo)][:terms]
    nT = len(seq)
    for ti, (wt, xt_) in enumerate(seq):
        for j in range(4):
            nc.tensor.matmul(
                ps, wt[:, 2 * j:2 * j + 2, :], xt_[:, 2 * j:2 * j + 2, n_slice],
                start=(ti == 0 and j == 0), stop=(ti == nT - 1 and j == 3),
                perf_mode=DRM)


def _mm3v(nc, ps, xhi, xlo, whi, wlo, t_slice, terms=3):
    """V-style: stationary x tile [128, 2, 128] slices, moving wv [128, 2, 256]."""
    seq = [(xhi, whi), (xlo, whi), (xhi, wlo)][:terms]
    nT = len(seq)
    for ti, (xt_, wt) in enumerate(seq):
        for j in range(4):
            nc.tensor.matmul(
                ps, xt_[:, 2 * j:2 * j + 2, t_slice], wt[:, 2 * j:2 * j + 2, :],
                start=(ti == 0 and j == 0), stop=(ti == nT - 1 and j == 3),
                perf_mode=DRM)


def _norm_to_at(nc, nm_pool, at, avx, tile_j, half, c):
    rc = nm_pool.tile([1, CH], BF16, tag="rc")
    with nc.allow_low_precision(reason="1/Z at bf16: 0.4% rel"):
        nc.vector.reciprocal(rc[:], avx[64:65, :])
    bs = nm_pool.tile([64, CH], BF16, tag="bs")
    nc.gpsimd.partition_broadcast(bs[:], rc[:])
    nc.vector.tensor_mul(
        at[64 * half:64 * half + 64, tile_j, c * CH:(c + 1) * CH],
        avx[0:64, :], bs[:])


def _attend(nc, tc, at, mk, mk8, eb, lnt, kts, qts, vt, vt8, g,
            es_pool, nm_pool, pp_s, pp_av):
    EXP = mybir.ActivationFunctionType.Exp
    for hp in range(2):  # head pairs; two heads run concurrently
        kt, qt = kts[hp], qts[hp]
        hl0, hl1 = 2 * hp, 2 * hp + 1
        tile_j = 2 * g + hp  # attnT e-tile index for this pair

        # Chunk A (own q [0:512)): bf16 es x bf16 v; both heads share one
        # psum.  Slots 0-3 diagonal-masked, slots 4-7 all-or-none per core
        # (killed via exp bias -80 from the per-core eb table).
        av0 = pp_av.tile([65, CH], F32, tag="av")
        av1 = pp_av.tile([65, CH], F32, tag="av")
        n = len(CHUNK_A)
        for si, (tt, mi) in enumerate(CHUNK_A):
            ps = pp_s.tile([128, 2 * CH], F32)
            for half, r0 in ((0, 0), (1, 64)):
                nc.tensor.matmul(
                    ps[:, half * CH:(half + 1) * CH],
                    kt[r0:r0 + 64, tt * 128:(tt + 1) * 128],
                    qt[r0:r0 + 64, 0:CH],
                    start=True, stop=True)
            es = es_pool.tile([128, 2 * CH], BF16, tag="esA")
            bias = 0.0 if mi is not None else eb[:, 0:1]
            nc.scalar.activation(es[:], ps[:], EXP, scale=SCALE, bias=bias)
            if mi is not None:
                nc.vector.tensor_mul(
                    es[:, 0:CH], es[:, 0:CH], mk[:, mi, :])
                nc.vector.tensor_mul(
                    es[:, CH:2 * CH], es[:, CH:2 * CH], mk[:, mi, :])
            nc.tensor.matmul(
                av0[:], vt[:, tt, hl0, :], es[:, 0:CH],
                start=(si == 0), stop=(si == n - 1))
            nc.tensor.matmul(
                av1[:], vt[:, tt, hl1, :], es[:, CH:2 * CH],
                start=(si == 0), stop=(si == n - 1))
        _norm_to_at(nc, nm_pool, at, av0, tile_j, 0, 0)
        _norm_to_at(nc, nm_pool, at, av1, tile_j, 1, 0)

        # Chunk B (own q [512:1024), >=512 keys attended): fp8 es x fp8 v
        # with DoubleRow AV over t-tile pairs, one head per psum.  Pairs
        # 2,3 are diagonal (masks), pairs 6,7 all-or-none (exp bias).
        for hl, r0 in ((hl0, 0), (hl1, 64)):
            av8 = pp_av.tile([65, CH], F32, tag="av")
            for pj in range(8):
                ps = pp_s.tile([128, 2, CH], F32)
                for i in (0, 1):
                    tt = 2 * pj + i
                    nc.tensor.matmul(
                        ps[:, i, :],
                        kt[r0:r0 + 64, tt * 128:(tt + 1) * 128],
                        qt[r0:r0 + 64, CH:2 * CH],
                        start=True, stop=True)
                es8 = es_pool.tile([128, 2, CH], F8, tag="esB")
                bias = eb[:, 1:2] if pj >= 6 else lnt[:, 0:1]
                nc.scalar.activation(es8[:], ps[:], EXP, scale=SCALE,
                                     bias=bias)
                if pj in (2, 3):
                    for i in (0, 1):
                        nc.vector.tensor_mul(
                            es8[:, i, :], es8[:, i, :],
                            mk8[:, 2 * (pj - 2) + i, :])
                nc.tensor.matmul(
                    av8[:], vt8[:, 2 * pj:2 * pj + 2, hl, 0:65], es8[:],
                    start=(pj == 0), stop=(pj == 7), perf_mode=DRM)
            _norm_to_at(nc, nm_pool, at, av8, tile_j, hl - hl0, 1)


def _phase1_attention(nc, tc, xhi, xlo, at, mk, mk8, eb, lnt, dram,
                      wkq_pool, wv_pool, pre):
    MUL = mybir.AluOpType.mult
    COPY = mybir.ActivationFunctionType.Copy

    def drain_qk(dst, ps):
        nc.vector.tensor_scalar(dst, ps, C_QK, None, MUL)

    def drain_v(vt, vt8, tt, ps, g):
        rea = ps[:, 0:256].rearrange("p (g d) -> p g d", g=4)
        if (tt // 4) % 2 == 0:
            # chunk-A reads only t-tiles 0-3 and 8-11 from the bf16 vt
            nc.vector.tensor_scalar(vt[:, tt, :, 0:64], rea, C_QK, None, MUL)
        nc.vector.tensor_scalar(vt8[:, tt, :, 0:64], rea, 2.0 ** -8,
                                None, MUL)

    with (
        tc.tile_pool(name="kt", bufs=4) as kt_pool,
        tc.tile_pool(name="qt", bufs=3) as qt_pool,
        tc.tile_pool(name="vt", bufs=1) as vt_pool,
        tc.tile_pool(name="vt8", bufs=2) as vt8_pool,
        tc.tile_pool(name="es", bufs=6) as es_pool,
        tc.tile_pool(name="norm", bufs=3) as nm_pool,
        tc.tile_pool(name="ps_kqv", bufs=2, space="PSUM") as pp_kqv,
        tc.tile_pool(name="ps_s", bufs=2, space="PSUM") as pp_s,
        tc.tile_pool(name="ps_av", bufs=2, space="PSUM") as pp_av,
    ):
        wk_d, wq_d, wv_d = dram["wk"], dram["wq"], dram["wv"]
        wklo_d, wqlo_d, wvlo_d = dram["wklo"], dram["wqlo"], dram["wvlo"]

        def load_w(dhi, dlo, p, shape):
            whi = wkq_pool.tile(shape, F8, tag="w")
            nc.sync.dma_start(whi[:], dhi[p])
            wlo = wkq_pool.tile(shape, F8E5, tag="w")
            nc.sync.dma_start(wlo[:], dlo[p])
            return whi, wlo

        for g in range(4):  # head groups of 4
            if g == 0:
                # Group 0 runs while x is still streaming in: interleave K
                # chunks with V tile blocks to ride the DMA stream.
                kt0 = kt_pool.tile([128, S], BF16)
                vt = vt_pool.tile([128, 16, 4, 65], BF16, tag="vt")
                nc.vector.memset(vt[:, :, :, 64:65], 1.0)
                vt8 = vt8_pool.tile([128, 16, 4, 72], F8)
                nc.vector.memset(vt8[:, :, :, 64:65], 16.0)
                (wk0h, wk0l), (wv0h, wv0l) = pre["wk", 0], pre["wv", 0]
                for cnk in range(4):
                    ps = pp_kqv.tile([128, CH], F32)
                    _mm3(nc, ps[:], wk0h, wk0l, xhi, xlo,
                         slice(cnk * CH, (cnk + 1) * CH),
                         terms=3 if cnk % 2 == 0 else 1)
                    drain_qk(kt0[:, cnk * CH:(cnk + 1) * CH], ps[:])
                    for tt in range(4 * cnk, 4 * cnk + 4):
                        ps = pp_kqv.tile([128, CH], F32)
                        _mm3v(nc, ps[:, 0:256], xhi, xlo, wv0h, wv0l,
                              slice(tt * 128, (tt + 1) * 128),
                              terms=3 if (tt // 4) % 2 == 0 else 1)
                        drain_v(vt, vt8, tt, ps, g)
                kts, qts = [kt0], []
                for pl in range(2):
                    if pl == 1:
                        kt1 = kt_pool.tile([128, S], BF16)
                        wk1h, wk1l = pre["wk", 1]
                        for cnk in range(4):
                            ps = pp_kqv.tile([128, CH], F32)
                            _mm3(nc, ps[:], wk1h, wk1l, xhi, xlo,
                                 slice(cnk * CH, (cnk + 1) * CH),
                                 terms=3 if cnk % 2 == 0 else 1)
                            drain_qk(kt1[:, cnk * CH:(cnk + 1) * CH], ps[:])
                        kts.append(kt1)
                    wqh, wql = pre["wq", pl]
                    qt = qt_pool.tile([128, QC], BF16)
                    for c in range(2):
                        ps = pp_kqv.tile([128, CH], F32)
                        _mm3(nc, ps[:], wqh, wql, xhi, xlo,
                             slice(c * CH, (c + 1) * CH),
                             terms=3 if c == 0 else 1)
                        drain_qk(qt[:, c * CH:(c + 1) * CH], ps[:])
                    qts.append(qt)
                _attend(nc, tc, at, mk, mk8, eb, lnt, kts, qts, vt, vt8, g,
                        es_pool, nm_pool, pp_s, pp_av)
                continue
            kts, qts = [], []
            for pl in range(2):
                p = 2 * g + pl
                wkh, wkl = pre.get(("wk", p)) or load_w(wk_d, wklo_d, p,
                                                        [128, ET, 128])
                kt = kt_pool.tile([128, S], BF16)
                for tcnk in range(4):
                    ps = pp_kqv.tile([128, CH], F32)
                    _mm3(nc, ps[:], wkh, wkl, xhi, xlo,
                         slice(tcnk * CH, (tcnk + 1) * CH),
                         terms=3 if tcnk % 2 == 0 else 1)
                    drain_qk(kt[:, tcnk * CH:(tcnk + 1) * CH], ps[:])
                kts.append(kt)

                wqh, wql = pre.get(("wq", p)) or load_w(wq_d, wqlo_d, p,
                                                        [128, ET, 128])
                qt = qt_pool.tile([128, QC], BF16)
                for c in range(2):
                    ps = pp_kqv.tile([128, CH], F32)
                    _mm3(nc, ps[:], wqh, wql, xhi, xlo,
                         slice(c * CH, (c + 1) * CH),
                         terms=3 if c == 0 else 1)
                    drain_qk(qt[:, c * CH:(c + 1) * CH], ps[:])
                qts.append(qt)

            if ("wv", g) in pre:
                wvh, wvl = pre["wv", g]
            else:
                wvh = wv_pool.tile([128, ET, 256], F8)
                nc.sync.dma_start(wvh[:], wv_d[g])
                wvl = wv_pool.tile([128, ET, 256], F8E5)
                nc.sync.dma_start(wvl[:], wvlo_d[g])
            vt = vt_pool.tile([128, 16, 4, 65], BF16, tag="vt")
            nc.vector.memset(vt[:, :, :, 64:65], 1.0)
            vt8 = vt8_pool.tile([128, 16, 4, 72], F8)
            nc.vector.memset(vt8[:, :, :, 64:65], 16.0)
            for tt in range(16):
                ps = pp_kqv.tile([128, CH], F32)
                _mm3v(nc, ps[:, 0:256], xhi, xlo, wvh, wvl,
                      slice(tt * 128, (tt + 1) * 128),
                      terms=3 if (tt // 4) % 2 == 0 else 1)
                drain_v(vt, vt8, tt, ps, g)

            _attend(nc, tc, at, mk, mk8, eb, lnt, kts, qts, vt, vt8, g,
                    es_pool, nm_pool, pp_s, pp_av)


def _phase2_proj(nc, tc, xt_d, xr, x1hi, x1lo, at, dram):
    """x1 = x + attn @ Wo + bo; residual x and Wo DMA'd here (bf16);
    x1 written to xr (bf16, residual) and quantized to x1hi/x1lo fp8."""
    ADD, MUL, SUB = (mybir.AluOpType.add, mybir.AluOpType.mult,
                     mybir.AluOpType.subtract)
    with (
        tc.tile_pool(name="xt", bufs=1) as xt_pool,
        tc.tile_pool(name="wo", bufs=1) as wo_pool,
        tc.tile_pool(name="bo", bufs=1) as bo_pool,
        tc.tile_pool(name="ps_y", bufs=6, space="PSUM") as pp_y,
    ):
        wo = wo_pool.tile([128, ET, ET, 128], BF16)
        for i in range(ET):
            nc.sync.dma_start(wo[:, i, :, :], dram["wo"][:, i, :, :])
        bo = bo_pool.tile([128, ET, 1], F32)
        nc.sync.dma_start(bo[:], dram["bo"][:])
        xt = xt_pool.tile([128, ET, QC], BF16)
        for j in range(ET):
            nc.sync.dma_start(xt[:, j, :], xt_d[:, j, :])
        for j in range(ET):
            for c in range(2):
                cs = slice(c * CH, (c + 1) * CH)
                ps = pp_y.tile([128, CH], F32)
                for i in range(ET):
                    nc.tensor.matmul(
                        ps[:], wo[:, i, j, :], at[:, i, cs],
                        start=(i == 0), stop=(i == ET - 1))
                nc.vector.scalar_tensor_tensor(
                    xr[:, j, cs], ps[:], bo[:, j, :], xt[:, j, cs],
                    op0=ADD, op1=ADD)
                nc.scalar.activation(
                    x1hi[:, j, cs], xr[:, j, cs],
                    mybir.ActivationFunctionType.Copy, scale=S_X1)
                nc.vector.scalar_tensor_tensor(
                    x1lo[:, j, cs], xr[:, j, cs], S_X1, x1hi[:, j, cs],
                    op0=MUL, op1=SUB)


def _phase3_ffn(nc, tc, xr, x1hi, x1lo, out_d, dram):
    ADD, MUL, SUB, MAX = (mybir.AluOpType.add, mybir.AluOpType.mult,
                          mybir.AluOpType.subtract, mybir.AluOpType.max)
    w1_d, w1lo_d = dram["w1"], dram["w1lo"]
    w2_d, w2lo_d = dram["w2"], dram["w2lo"]
    with (
        tc.tile_pool(name="hT", bufs=2) as h_pool,
        tc.tile_pool(name="usb", bufs=3) as u_pool,
        tc.tile_pool(name="w1s", bufs=6) as w1_pool,
        tc.tile_pool(name="w2s", bufs=2) as w2_pool,
        tc.tile_pool(name="b12", bufs=1) as b12_pool,
        tc.tile_pool(name="osb", bufs=3) as o_pool,
        tc.tile_pool(name="ps_h", bufs=4, space="PSUM") as pp_h,
        tc.tile_pool(name="ps_f", bufs=4, space="PSUM") as pp_f,
    ):
        b1c = b12_pool.tile([128, HT, 1], F32, tag="b1c")
        nc.sync.dma_start(b1c[:], dram["b1c"][:])
        b2 = b12_pool.tile([128, ET, 1], F32, tag="b2")
        nc.sync.dma_start(b2[:], dram["b2"][:])
        hhi0 = h_pool.tile([128, HT, CH], F8, tag="hhi")
        hhi1 = h_pool.tile([128, HT, CH], F8, tag="hhi")
        hlo0 = h_pool.tile([128, HT, CH], F8E5, tag="hlo")
        hlo1 = h_pool.tile([128, HT, CH], F8E5, tag="hlo")
        hhi = [hhi0, hhi1]
        hlo = [hlo0, hlo1]
        # preload the first FFN2 weight tiles ahead of the 64 w1 DMAs
        w2t0h = w2_pool.tile([128, HT, 128], F8)
        nc.sync.dma_start(w2t0h[:], w2_d[0])
        w2t0l = w2_pool.tile([128, HT, 128], F8E5)
        nc.sync.dma_start(w2t0l[:], w2lo_d[0])
        for t in range(HT):
            w1th = w1_pool.tile([128, ET, 128], F8)
            nc.sync.dma_start(w1th[:], w1_d[t])
            w1tl = w1_pool.tile([128, ET, 128], F8E5)
            nc.sync.dma_start(w1tl[:], w1lo_d[t])
            for c in range(2):
                ps = pp_h.tile([128, CH], F32)
                _mm3(nc, ps[:], w1th, w1tl, x1hi, x1lo,
                     slice(c * CH, (c + 1) * CH))
                # u = relu(ps + b1*S_X1*S_W1) = (S_X1*S_W1)*h  (ACT, exact)
                u = u_pool.tile([128, CH], BF16)
                nc.scalar.activation(
                    u[:], ps[:], mybir.ActivationFunctionType.Relu,
                    bias=b1c[:, t, :])
                if t % 2 == 0:
                    nc.scalar.activation(
                        hhi[c][:, t, :], u[:],
                        mybir.ActivationFunctionType.Copy, scale=C_H)
                else:
                    # split the h_hi quantize across ACT/DVE: ACT otherwise
                    # runs 78us in FFN1, nearly matching the 82us of PE work
                    nc.vector.tensor_scalar(
                        hhi[c][:, t, :], u[:], C_H, None, MUL)
                nc.vector.scalar_tensor_tensor(
                    hlo[c][:, t, :], u[:], C_H, hhi[c][:, t, :],
                    op0=MUL, op1=SUB)
        for j in range(ET):
            if j == 0:
                w2th, w2tl = w2t0h, w2t0l
            else:
                w2th = w2_pool.tile([128, HT, 128], F8)
                nc.sync.dma_start(w2th[:], w2_d[j])
                w2tl = w2_pool.tile([128, HT, 128], F8E5)
                nc.sync.dma_start(w2tl[:], w2lo_d[j])
            for c in range(2):
                ps = pp_f.tile([128, CH], F32)
                seq = [(w2th, hhi[c]), (w2tl, hhi[c]), (w2th, hlo[c])]
                for ti, (wt, ht) in enumerate(seq):
                    for tp in range(HT // 2):
                        nc.tensor.matmul(
                            ps[:], wt[:, 2 * tp:2 * tp + 2, :],
                            ht[:, 2 * tp:2 * tp + 2, :],
                            start=(ti == 0 and tp == 0),
                            stop=(ti == 2 and tp == HT // 2 - 1),
                            perf_mode=DRM)
                tb = o_pool.tile([128, CH], BF16, tag="tb")
                nc.vector.tensor_scalar(
                    tb[:], ps[:], C_F2, b2[:, j, :], MUL, ADD)
                ot = o_pool.tile([128, CH], BF16, tag="ot")
                nc.vector.tensor_tensor(
                    ot[:], tb[:], xr[:, j, c * CH:(c + 1) * CH], op=ADD)
                nc.sync.dma_start(out_d[j][:, c * CH:(c + 1) * CH], ot[:])


def build_nc(reps=1, phases=(1, 2, 3)):
    nc = bacc.Bacc("TRN2", target_bir_lowering=False, debug=False, num_devices=8)

    dram = {}
    dram["xT"] = nc.declare_dram_parameter("xT", [128, ET, QC], BF16, isOutput=False)
    dram["xhi"] = nc.declare_dram_parameter("xhi", [128, ET, S], F8, isOutput=False)
    dram["xlo"] = nc.declare_dram_parameter("xlo", [128, ET, S], F8E5, isOutput=False)
    dram["wq"] = nc.declare_dram_parameter("wq", [8, 128, ET, 128], F8, isOutput=False)
    dram["wqlo"] = nc.declare_dram_parameter("wqlo", [8, 128, ET, 128], F8E5, isOutput=False)
    dram["wk"] = nc.declare_dram_parameter("wk", [8, 128, ET, 128], F8, isOutput=False)
    dram["wklo"] = nc.declare_dram_parameter("wklo", [8, 128, ET, 128], F8E5, isOutput=False)
    dram["wv"] = nc.declare_dram_parameter("wv", [4, 128, ET, 256], F8, isOutput=False)
    dram["wvlo"] = nc.declare_dram_parameter("wvlo", [4, 128, ET, 256], F8E5, isOutput=False)
    dram["wo"] = nc.declare_dram_parameter("wo", [128, ET, ET, 128], BF16, isOutput=False)
    dram["w1"] = nc.declare_dram_parameter("w1", [HT, 128, ET, 128], F8, isOutput=False)
    dram["w1lo"] = nc.declare_dram_parameter("w1lo", [HT, 128, ET, 128], F8E5, isOutput=False)
    dram["w2"] = nc.declare_dram_parameter("w2", [ET, 128, HT, 128], F8, isOutput=False)
    dram["w2lo"] = nc.declare_dram_parameter("w2lo", [ET, 128, HT, 128], F8E5, isOutput=False)
    dram["bo"] = nc.declare_dram_parameter("bo", [128, ET, 1], F32, isOutput=False)
    dram["b1s"] = nc.declare_dram_parameter("b1s", [128, HT, 1], F32, isOutput=False)
    dram["b1c"] = nc.declare_dram_parameter("b1c", [128, HT, 1], F32, isOutput=False)
    dram["b2"] = nc.declare_dram_parameter("b2", [128, ET, 1], F32, isOutput=False)
    dram["masks"] = nc.declare_dram_parameter(
        "masks", [128, N_MASKS, CH], BF16, isOutput=False)
    dram["masks8"] = nc.declare_dram_parameter(
        "masks8", [128, N_MASKS, CH], F8, isOutput=False)
    dram["ebias"] = nc.declare_dram_parameter(
        "ebias", [128, 2], F32, isOutput=False)
    out_d = nc.declare_dram_parameter("outT", [ET, 128, QC], BF16, isOutput=True)

    with tile.TileContext(nc) as tc:
        for _rep in range(reps):
            with (
                tc.tile_pool(name="xq", bufs=1) as xq_pool,
                tc.tile_pool(name="xr", bufs=1) as xr_pool,
                tc.tile_pool(name="x1q", bufs=1) as x1q_pool,
            ):
                xhi = xq_pool.tile([128, ET, S], F8, tag="xhi")
                xlo = xq_pool.tile([128, ET, S], F8E5, tag="xlo")
                xr = xr_pool.tile([128, ET, QC], BF16)
                x1hi = x1q_pool.tile([128, ET, QC], F8, tag="x1hi")
                x1lo = x1q_pool.tile([128, ET, QC], F8E5, tag="x1lo")
                with (
                    tc.tile_pool(name="masks", bufs=1) as mk_pool,
                    tc.tile_pool(name="wkq", bufs=8) as wkq_pool,
                    tc.tile_pool(name="wv", bufs=2) as wv_pool,
                    tc.tile_pool(name="attnT", bufs=1) as at_pool,
                ):
                    mk = mk_pool.tile([128, N_MASKS, CH], BF16, tag="mk")
                    mk8 = mk_pool.tile([128, N_MASKS, CH], F8, tag="mk8")
                    eb = mk_pool.tile([128, 2], F32, tag="eb")
                    lnt = mk_pool.tile([128, 1], F32, tag="lnt")
                    nc.vector.memset(lnt[:], LN_SES)
                    # DMA priority order: first K matmuls need only x chunk
                    # 0 + wk plane 0, so emit those first; masks and
                    # phase-2-only inputs last.
                    pre = {}

                    def prew(key, p, dhi, dlo, shape):
                        whi = wkq_pool.tile(shape, F8, tag="w")
                        nc.sync.dma_start(whi[:], dram[dhi][p])
                        wlo = wkq_pool.tile(shape, F8E5, tag="w")
                        nc.sync.dma_start(wlo[:], dram[dlo][p])
                        pre[key, p] = (whi, wlo)

                    prew("wk", 0, "wk", "wklo", [128, ET, 128])
                    for et in range(ET):
                        nc.sync.dma_start(
                            xhi[:, et, 0:CH], dram["xhi"][:, et, 0:CH])
                        nc.sync.dma_start(
                            xlo[:, et, 0:CH], dram["xlo"][:, et, 0:CH])
                    wv0h = wv_pool.tile([128, ET, 256], F8)
                    nc.sync.dma_start(wv0h[:], dram["wv"][0])
                    wv0l = wv_pool.tile([128, ET, 256], F8E5)
                    nc.sync.dma_start(wv0l[:], dram["wvlo"][0])
                    pre["wv", 0] = (wv0h, wv0l)
                    nc.sync.dma_start(
                        xhi[:, :, CH:2 * CH], dram["xhi"][:, :, CH:2 * CH])
                    nc.sync.dma_start(
                        xlo[:, :, CH:2 * CH], dram["xlo"][:, :, CH:2 * CH])
                    prew("wq", 0, "wq", "wqlo", [128, ET, 128])
                    nc.sync.dma_start(
                        xhi[:, :, 2 * CH:4 * CH], dram["xhi"][:, :, 2 * CH:4 * CH])
                    nc.sync.dma_start(
                        xlo[:, :, 2 * CH:4 * CH], dram["xlo"][:, :, 2 * CH:4 * CH])
                    prew("wk", 1, "wk", "wklo", [128, ET, 128])
                    nc.sync.dma_start(mk[:], dram["masks"][:])
                    nc.sync.dma_start(eb[:], dram["ebias"][:])
                    prew("wq", 1, "wq", "wqlo", [128, ET, 128])
                    nc.sync.dma_start(mk8[:], dram["masks8"][:])
                    at = at_pool.tile([128, ET, QC], BF16)
                    if 1 in phases:
                        _phase1_attention(nc, tc, xhi, xlo, at, mk, mk8,
                                          eb, lnt, dram,
                                          wkq_pool, wv_pool, pre)
                    if 2 in phases:
                        _phase2_proj(nc, tc, dram["xT"], xr, x1hi, x1lo,
                                     at, dram)
                if 3 in phases:
                    _phase3_ffn(nc, tc, xr, x1hi, x1lo, out_d, dram)

    nc.compile()
    return nc


def _qsel(half):
    if half == 0:
        return np.concatenate([np.arange(0, 512), np.arange(1536, 2048)])
    return np.arange(512, 1536)


def make_masks():
    """Universal diagonal keep-masks: tri_i[j, c] = (128*i + j <= c).

    Both chunk-A slots 0-3 and chunk-B diagonal tiles (t-tiles 4-7 vs own
    q [512:1024)) reduce to the same four relative patterns because own-q
    indices are sorted ascending."""
    j = np.arange(128)[:, None]
    c = np.arange(CH)[None, :]
    m = np.stack([(128 * i + j <= c).astype(np.float32) for i in range(4)])
    m = np.ascontiguousarray(m.transpose(1, 0, 2))
    return m.astype(ml_dtypes.bfloat16), m.astype(NPF8)


def make_ebias(half):
    """Per-core exp-bias table [128, 2] f32.

    col 0: chunk-A slots 4-7 (t-tiles 8-11 = other qA): half0 -> all-invalid
    (-80 => es=0), half1 -> all-valid (0).
    col 1: chunk-B pairs 6,7 (t-tiles 12-15 = other qB): half0 -> valid
    (ln 16), half1 -> invalid (ln 16 - 80)."""
    a = -80.0 if half == 0 else 0.0
    b = LN_SES if half == 0 else LN_SES - 80.0
    return np.tile(np.array([[a, b]], np.float32), (128, 1))


def _hilo(w, s):
    """Quantize w*s into fp8e4m3 hi + fp8e5m2 lo residual."""
    ws = np.asarray(w, np.float32) * np.float32(s)
    hi = np.clip(ws, -240.0, 240.0).astype(NPF8)
    lo = (ws - hi.astype(np.float32)).astype(NPF8E5)
    return np.ascontiguousarray(hi), np.ascontiguousarray(lo)


def prep_shared(Wq, Wk, Wv, Wo, bo, W1, b1, W2, b2):
    f = np.float32
    wq = np.stack([Wq[2 * p:2 * p + 2].transpose(1, 0, 2).reshape(E, 128)
                   .reshape(ET, 128, 128).transpose(1, 0, 2) for p in range(8)])
    wk = np.stack([Wk[2 * p:2 * p + 2].transpose(1, 0, 2).reshape(E, 128)
                   .reshape(ET, 128, 128).transpose(1, 0, 2) for p in range(8)])
    wv = np.stack([Wv[4 * g:4 * g + 4].transpose(1, 0, 2).reshape(E, 256)
                   .reshape(ET, 128, 256).transpose(1, 0, 2) for g in range(4)])
    wo = Wo.reshape(ET, 128, ET, 128).transpose(1, 0, 2, 3)
    w1 = W1.reshape(ET, 128, HT, 128).transpose(2, 1, 0, 3)
    w2 = W2.reshape(HT, 128, ET, 128).transpose(2, 1, 0, 3)
    wqh, wql = _hilo(wq, S_W)
    wkh, wkl = _hilo(wk, S_W)
    wvh, wvl = _hilo(wv, S_W)
    w1h, w1l = _hilo(w1, S_W1)
    w2h, w2l = _hilo(w2, S_W2)
    b1t = np.ascontiguousarray(b1.reshape(HT, 128, 1).transpose(1, 0, 2)).astype(f)
    return {
        "wq": wqh, "wqlo": wql,
        "wk": wkh, "wklo": wkl,
        "wv": wvh, "wvlo": wvl,
        "wo": np.ascontiguousarray(wo).astype(ml_dtypes.bfloat16),
        "w1": w1h, "w1lo": w1l,
        "w2": w2h, "w2lo": w2l,
        "bo": np.ascontiguousarray(bo.reshape(ET, 128, 1).transpose(1, 0, 2)).astype(f),
        "b1s": b1t * f(S_H),
        "b1c": b1t * f(S_X1 * S_W1),
        "b2": np.ascontiguousarray(b2.reshape(ET, 128, 1).transpose(1, 0, 2)).astype(f),
    }


def make_in_maps(x, Wq, Wk, Wv, Wo, bo, W1, b1, W2, b2):
    shared = prep_shared(Wq, Wk, Wv, Wo, bo, W1, b1, W2, b2)
    mk_bf, mk_f8 = make_masks()
    ebias = [make_ebias(half) for half in range(2)]
    in_maps = []
    for core in range(8):
        b, half = core // 2, core % 2
        own = _qsel(half)
        torder = np.concatenate([own, _qsel(1 - half)])
        xTc = np.ascontiguousarray(np.asarray(x[b]).T[:, torder]
                                   .reshape(ET, 128, S).transpose(1, 0, 2))
        xhi, xlo = _hilo(xTc, S_X)
        in_maps.append({"xT": np.ascontiguousarray(xTc[:, :, 0:QC])
                        .astype(ml_dtypes.bfloat16),
                        "xhi": xhi, "xlo": xlo,
                        "masks": mk_bf, "masks8": mk_f8,
                        "ebias": ebias[half], **shared})
    return in_maps


def kernel(**inputs):
    global LAST_RESULTS
    if "nc" not in _CACHE:
        _CACHE["nc"] = build_nc()
    nc = _CACHE["nc"]
    in_maps = make_in_maps(
        inputs["x"], inputs["Wq"], inputs["Wk"], inputs["Wv"], inputs["Wo"],
        inputs["bo"], inputs["W1"], inputs["b1"], inputs["W2"], inputs["b2"])
    res = run_bass_kernel_spmd(nc, in_maps, list(range(8)))
    LAST_RESULTS = res
    out = np.empty((B, S, E), dtype=np.float32)
    for core in range(8):
        b, half = core // 2, core % 2
        outT = res.results[core]["outT"].reshape(E, QC).astype(np.float32)
        out[b, _qsel(half), :] = outT.T
    return out

